# revision 6
# baseline (speedup 1.0000x reference)
"""Trainium2 Bass kernel for ConditionalFeedForward (MoE routed FFN).

Problem: M=2048 tokens, D=1024, I=2048, E=8 experts, TOPK=2.
out[t, s] = FFN_{e}(x[t]) with e = expert_indices[t, s], where
FFN_e(x) = (silu(x @ w1_e.T) * (x @ w3_e.T)) @ w2_e.T  (w13 = [w1; w3]).

Strategy (expert parallelism, 8 experts -> 8 cores):
 - Host routes (token, slot) pairs to the core owning the selected expert,
   pads each core's token batch to a common capacity C, and transposes
   activations so features live on SBUF partitions and tokens on the free
   dim.  No device collectives needed: the "all-to-all" is a host gather
   and scatter around one SPMD kernel launch.
 - Per core: hT = w13_e @ x_eT via PE (fp32r, full rate at free dim >=256),
   g = silu(h1) * h3 on ACT+DVE, outT = w2_e.T-contract on PE, DMA out.
 - Weights stream through SBUF once (24 MB/core), pre-tiled on the host
   into the exact [128, k*128] lhsT layout the tensor engine wants.
"""

import os

import numpy as np

import concourse.bass as bass
import concourse.tile as tile
from concourse import bacc, mybir
from concourse.bass_utils import run_bass_kernel_spmd

M, D, I, E, TOPK = 2048, 1024, 2048, 8, 2
P = 128
KD = D // P            # 8   k-tiles over D (mm1 contraction)
NI2 = (2 * I) // P     # 32  n-tiles over 2I (mm1 output rows)
NPAIR = NI2 // 2       # 16  (x1, x3) pairs
KI = I // P            # 16  k-tiles over I (mm2 contraction)
ND = D // P            # 8   d-tiles over D (mm2 output rows)

F32 = mybir.dt.float32
F32R = mybir.dt.float32r

# exec time of the most recent launch (ns), populated when BASS_TRACE=1
LAST_EXEC_TIME_NS = None

_program_cache = {}


def _chunks_for(C):
    """Split C token-columns into matmul moving-dim chunks (<=512 each)."""
    n_ch = -(-C // 512)
    base = -(-C // (n_ch * 32)) * 32
    chunks = []
    off = 0
    while off < C:
        cn = min(base, C - off)
        chunks.append((off, cn))
        off += cn
    return tuple(chunks)


def _build_program(C):
    chunks = _chunks_for(C)
    nc = bacc.Bacc("TRN2", target_bir_lowering=False, debug=False, num_devices=E)

    xT_d = nc.dram_tensor("xT", (KD, P, C), F32R, kind="ExternalInput").ap()
    w13_d = nc.dram_tensor("w13t", (NI2, P, KD * P), F32R, kind="ExternalInput").ap()
    w2_d = nc.dram_tensor("w2t", (ND, P, KI * P), F32R, kind="ExternalInput").ap()
    out_d = nc.dram_tensor("outT", (ND, P, C), F32, kind="ExternalOutput").ap()

    with tile.TileContext(nc) as tc:
        with (
            tc.tile_pool(name="xg", bufs=1) as xg_pool,
            tc.tile_pool(name="wt", bufs=3) as w_pool,
            tc.tile_pool(name="tmp", bufs=4) as tmp_pool,
            tc.tile_pool(name="ps", bufs=4, space="PSUM") as ps_pool,
        ):
            x_tiles = []
            for k in range(KD):
                xt = xg_pool.tile([P, C], F32R, tag=f"x{k}", name=f"x{k}")
                nc.sync.dma_start(xt[:], xT_d[k])
                x_tiles.append(xt)

            g_tiles = [
                xg_pool.tile([P, C], F32R, tag=f"g{ki}", name=f"g{ki}")
                for ki in range(KI)
            ]

            # ---- mm1 + silu*gate: process (x1, x3) row-block pairs ----
            for pr in range(NPAIR):
                wA = w_pool.tile([P, KD * P], F32R, tag="w13", name="w13")
                nc.sync.dma_start(wA[:], w13_d[pr])
                wB = w_pool.tile([P, KD * P], F32R, tag="w13", name="w13")
                nc.sync.dma_start(wB[:], w13_d[pr + NPAIR])
                for c0, cn in chunks:
                    psA = ps_pool.tile([P, 512], F32, tag="ps1", name="ps1")[:, :cn]
                    psB = ps_pool.tile([P, 512], F32, tag="ps1", name="ps1")[:, :cn]
                    for k in range(KD):
                        nc.tensor.matmul(
                            psA,
                            wA[:, k * P : (k + 1) * P],
                            x_tiles[k][:, c0 : c0 + cn],
                            start=(k == 0),
                            stop=(k == KD - 1),
                        )
                    for k in range(KD):
                        nc.tensor.matmul(
                            psB,
                            wB[:, k * P : (k + 1) * P],
                            x_tiles[k][:, c0 : c0 + cn],
                            start=(k == 0),
                            stop=(k == KD - 1),
                        )
                    s = tmp_pool.tile([P, 512], F32, tag="s", name="s")[:, :cn]
                    nc.scalar.activation(s, psA, mybir.ActivationFunctionType.Silu)
                    nc.vector.tensor_mul(
                        out=g_tiles[pr][:, c0 : c0 + cn],
                        in0=s,
                        in1=psB,
                    )

            # ---- mm2: outT[d-block] = sum_ki w2T-tile @ g ----
            for d in range(ND):
                wD = w_pool.tile([P, KI * P], F32R, tag="w2", name="w2")
                nc.sync.dma_start(wD[:], w2_d[d])
                for c0, cn in chunks:
                    psO = ps_pool.tile([P, 512], F32, tag="ps2", name="ps2")[:, :cn]
                    for ki in range(KI):
                        nc.tensor.matmul(
                            psO,
                            wD[:, ki * P : (ki + 1) * P],
                            g_tiles[ki][:, c0 : c0 + cn],
                            start=(ki == 0),
                            stop=(ki == KI - 1),
                        )
                    ot = tmp_pool.tile([P, 512], F32, tag="o", name="o")[:, :cn]
                    nc.vector.tensor_copy(ot, psO)
                    nc.sync.dma_start(out_d[d][:, c0 : c0 + cn], ot)

    nc.compile()
    return nc


def _get_program(C):
    if C not in _program_cache:
        _program_cache[C] = _build_program(C)
    return _program_cache[C]


def _ensure_ntff_hook():
    """Provide antenv.axon_hooks if the image lacks it, so trace=True works."""
    import sys
    import types

    try:
        import antenv.axon_hooks  # noqa: F401

        return
    except ImportError:
        pass
    try:
        import antenv
        from trn_agent_boot.trn_boot import _ntff_profile_via_ctypes

        mod = types.ModuleType("antenv.axon_hooks")
        state = {"hook": None}
        mod.set_axon_ntff_profile_hook = lambda h: state.__setitem__("hook", h)
        mod.get_axon_ntff_profile_hook = lambda: state["hook"]
        sys.modules["antenv.axon_hooks"] = mod
        antenv.axon_hooks = mod
        mod.set_axon_ntff_profile_hook(
            _ntff_profile_via_ctypes("/opt/axon/libaxon_pjrt.so")
        )
    except Exception:
        pass


def kernel(x, w13, w2, expert_indices):
    global LAST_EXEC_TIME_NS
    x = np.asarray(x, dtype=np.float32)
    w13 = np.asarray(w13, dtype=np.float32)
    w2 = np.asarray(w2, dtype=np.float32)
    idx = np.asarray(expert_indices)
    out_idx_dtype = idx.dtype
    idx32 = idx.astype(np.int64)

    m, d_model = x.shape
    e, two_i, _ = w13.shape
    inter = w2.shape[2]
    topk = idx.shape[1]
    assert (m, d_model, e, two_i, inter, topk) == (M, D, E, 2 * I, I, TOPK)

    # ---- host routing: group (token, slot) pairs by expert ----
    flat_e = idx32.reshape(-1)                       # [M*TOPK]
    order = np.argsort(flat_e, kind="stable")        # pair ids grouped by expert
    counts = np.bincount(flat_e, minlength=E)
    starts = np.concatenate([[0], np.cumsum(counts)])
    C = max(288, int(-(-counts.max() // 32) * 32))

    nc = _get_program(C)

    in_maps = []
    for ei in range(E):
        pair_ids = order[starts[ei] : starts[ei + 1]]
        tok_ids = pair_ids // topk
        cnt = len(tok_ids)

        xg = np.zeros((C, D), dtype=np.float32)
        xg[:cnt] = x[tok_ids]
        xT = np.ascontiguousarray(xg.T).reshape(KD, P, C)

        A4 = w13[ei].reshape(NI2, P, KD, P)          # [n, c, k, p]
        w13t = np.ascontiguousarray(A4.transpose(0, 3, 2, 1)).reshape(NI2, P, KD * P)
        B4 = w2[ei].reshape(ND, P, KI, P)            # [d, c, ki, p]
        w2t = np.ascontiguousarray(B4.transpose(0, 3, 2, 1)).reshape(ND, P, KI * P)

        in_maps.append({"xT": xT, "w13t": w13t, "w2t": w2t})

    trace = bool(os.environ.get("BASS_TRACE"))
    if trace:
        _ensure_ntff_hook()
    res = run_bass_kernel_spmd(nc, in_maps, core_ids=list(range(E)), trace=trace)
    LAST_EXEC_TIME_NS = res.exec_time_ns

    # ---- host scatter: un-permute per-expert outputs back to (token, slot) ----
    out_pairs = np.empty((M * TOPK, D), dtype=np.float32)
    for ei in range(E):
        pair_ids = order[starts[ei] : starts[ei + 1]]
        outT = res.results[ei]["outT"].reshape(D, C)
        out_pairs[pair_ids] = outT[:, : len(pair_ids)].T

    del out_idx_dtype
    return out_pairs.reshape(M, TOPK, D)


# revision 9
# speedup vs baseline: 1.1228x; 1.1228x over previous
"""Trainium2 Bass kernel for ConditionalFeedForward (MoE routed FFN).

Problem: M=2048 tokens, D=1024, I=2048, E=8 experts, TOPK=2.
out[t, s] = FFN_{e}(x[t]) with e = expert_indices[t, s], where
FFN_e(x) = (silu(x @ w1_e.T) * (x @ w3_e.T)) @ w2_e.T  (w13 = [w1; w3]).

Strategy (expert parallelism, 8 experts -> 8 cores):
 - Host routes (token, slot) pairs to the core owning the selected expert,
   pads each core's token batch to a common capacity C, and transposes
   activations so features live on SBUF partitions and tokens on the free
   dim.  No device collectives needed: the "all-to-all" is a host gather
   and scatter around one SPMD kernel launch.
 - Per core: hT = w13_e @ x_eT via PE (fp32r, full rate at free dim >=256),
   g = silu(h1) * h3 on ACT+DVE, outT = w2_e.T-contract on PE, DMA out.
 - Weights stream through SBUF once (24 MB/core), pre-tiled on the host
   into the exact [128, k*128] lhsT layout the tensor engine wants.
"""

import os

import numpy as np

import concourse.bass as bass
import concourse.tile as tile
from concourse import bacc, mybir
from concourse.bass_utils import run_bass_kernel_spmd

M, D, I, E, TOPK = 2048, 1024, 2048, 8, 2
P = 128
KD = D // P            # 8   k-tiles over D (mm1 contraction)
NI2 = (2 * I) // P     # 32  n-tiles over 2I (mm1 output rows)
NPAIR = NI2 // 2       # 16  (x1, x3) pairs
KI = I // P            # 16  k-tiles over I (mm2 contraction)
ND = D // P            # 8   d-tiles over D (mm2 output rows)

F32 = mybir.dt.float32
F32R = mybir.dt.float32r

# exec time of the most recent launch (ns), populated when BASS_TRACE=1
LAST_EXEC_TIME_NS = None

_program_cache = {}


def _chunks_for(C):
    """Split C token-columns into matmul moving-dim chunks (<=512 each)."""
    n_ch = -(-C // 512)
    base = -(-C // (n_ch * 32)) * 32
    chunks = []
    off = 0
    while off < C:
        cn = min(base, C - off)
        chunks.append((off, cn))
        off += cn
    return tuple(chunks)


def _build_program(C):
    chunks = _chunks_for(C)
    nc = bacc.Bacc("TRN2", target_bir_lowering=False, debug=False, num_devices=E)

    xT_d = nc.dram_tensor("xT", (KD, P, C), F32R, kind="ExternalInput").ap()
    w13_d = nc.dram_tensor("w13t", (NI2, P, KD * P), F32R, kind="ExternalInput").ap()
    w2_d = nc.dram_tensor("w2t", (ND, P, KI * P), F32R, kind="ExternalInput").ap()
    out_d = nc.dram_tensor("outT", (ND, P, C), F32, kind="ExternalOutput").ap()

    with tile.TileContext(nc) as tc:
        with (
            tc.tile_pool(name="xg", bufs=1) as xg_pool,
            tc.tile_pool(name="wt", bufs=3) as w_pool,
            tc.tile_pool(name="tmp", bufs=4) as tmp_pool,
            tc.tile_pool(name="ps", bufs=8, space="PSUM") as ps_pool,
        ):
            w13_buf = {}

            def issue_w13(pr):
                wA = w_pool.tile([P, KD * P], F32R, tag="w13", name="w13", bufs=4)
                nc.sync.dma_start(wA[:], w13_d[pr])
                wB = w_pool.tile([P, KD * P], F32R, tag="w13", name="w13", bufs=4)
                nc.sync.dma_start(wB[:], w13_d[pr + NPAIR])
                w13_buf[pr] = (wA, wB)

            w2_buf = {}

            def issue_w2(d):
                wD = w_pool.tile([P, KI * P], F32R, tag="w2", name="w2", bufs=2)
                nc.sync.dma_start(wD[:], w2_d[d])
                w2_buf[d] = wD

            # pair-0 weights first so the PE can start as soon as x lands
            issue_w13(0)

            x_tiles = []
            for k in range(KD):
                xt = xg_pool.tile([P, C], F32R, tag=f"x{k}", name=f"x{k}")
                nc.sync.dma_start(xt[:], xT_d[k])
                x_tiles.append(xt)
            issue_w13(1)

            g_tiles = [
                xg_pool.tile([P, C], F32R, tag=f"g{ki}", name=f"g{ki}")
                for ki in range(KI)
            ]

            # ---- mm1 + silu*gate: process (x1, x3) row-block pairs ----
            for pr in range(NPAIR):
                if pr + 2 < NPAIR:
                    issue_w13(pr + 2)
                elif pr + 2 - NPAIR < min(2, ND):
                    issue_w2(pr + 2 - NPAIR)
                wA, wB = w13_buf.pop(pr)
                for c0, cn in chunks:
                    psA = ps_pool.tile([P, 512], F32, tag="ps", name="ps")[:, :cn]
                    psB = ps_pool.tile([P, 512], F32, tag="ps", name="ps")[:, :cn]
                    for k in range(KD):
                        nc.tensor.matmul(
                            psA,
                            wA[:, k * P : (k + 1) * P],
                            x_tiles[k][:, c0 : c0 + cn],
                            start=(k == 0),
                            stop=(k == KD - 1),
                        )
                    for k in range(KD):
                        nc.tensor.matmul(
                            psB,
                            wB[:, k * P : (k + 1) * P],
                            x_tiles[k][:, c0 : c0 + cn],
                            start=(k == 0),
                            stop=(k == KD - 1),
                        )
                    s = tmp_pool.tile([P, 512], F32, tag="s", name="s")[:, :cn]
                    nc.scalar.activation(s, psA, mybir.ActivationFunctionType.Silu)
                    nc.vector.tensor_mul(
                        out=g_tiles[pr][:, c0 : c0 + cn],
                        in0=s,
                        in1=psB,
                    )

            # ---- mm2: outT[d-block] = sum_ki w2T-tile @ g ----
            for d in range(ND):
                if d + 2 < ND:
                    issue_w2(d + 2)
                wD = w2_buf.pop(d)
                for c0, cn in chunks:
                    psO = ps_pool.tile([P, 512], F32, tag="ps", name="ps")[:, :cn]
                    for ki in range(KI):
                        nc.tensor.matmul(
                            psO,
                            wD[:, ki * P : (ki + 1) * P],
                            g_tiles[ki][:, c0 : c0 + cn],
                            start=(ki == 0),
                            stop=(ki == KI - 1),
                        )
                    ot = tmp_pool.tile([P, 512], F32, tag="o", name="o")[:, :cn]
                    nc.vector.tensor_copy(ot, psO)
                    nc.sync.dma_start(out_d[d][:, c0 : c0 + cn], ot)

    nc.compile()
    return nc


def _get_program(C):
    if C not in _program_cache:
        _program_cache[C] = _build_program(C)
    return _program_cache[C]


def _ensure_ntff_hook():
    """Provide antenv.axon_hooks if the image lacks it, so trace=True works."""
    import sys
    import types

    try:
        import antenv.axon_hooks  # noqa: F401

        return
    except ImportError:
        pass
    try:
        import antenv
        from trn_agent_boot.trn_boot import _ntff_profile_via_ctypes

        mod = types.ModuleType("antenv.axon_hooks")
        state = {"hook": None}
        mod.set_axon_ntff_profile_hook = lambda h: state.__setitem__("hook", h)
        mod.get_axon_ntff_profile_hook = lambda: state["hook"]
        sys.modules["antenv.axon_hooks"] = mod
        antenv.axon_hooks = mod
        mod.set_axon_ntff_profile_hook(
            _ntff_profile_via_ctypes("/opt/axon/libaxon_pjrt.so")
        )
    except Exception:
        pass


def kernel(x, w13, w2, expert_indices):
    global LAST_EXEC_TIME_NS
    x = np.asarray(x, dtype=np.float32)
    w13 = np.asarray(w13, dtype=np.float32)
    w2 = np.asarray(w2, dtype=np.float32)
    idx = np.asarray(expert_indices)
    out_idx_dtype = idx.dtype
    idx32 = idx.astype(np.int64)

    m, d_model = x.shape
    e, two_i, _ = w13.shape
    inter = w2.shape[2]
    topk = idx.shape[1]
    assert (m, d_model, e, two_i, inter, topk) == (M, D, E, 2 * I, I, TOPK)

    # ---- host routing: group (token, slot) pairs by expert ----
    flat_e = idx32.reshape(-1)                       # [M*TOPK]
    order = np.argsort(flat_e, kind="stable")        # pair ids grouped by expert
    counts = np.bincount(flat_e, minlength=E)
    starts = np.concatenate([[0], np.cumsum(counts)])
    C = max(288, int(-(-counts.max() // 32) * 32))

    nc = _get_program(C)

    in_maps = []
    for ei in range(E):
        pair_ids = order[starts[ei] : starts[ei + 1]]
        tok_ids = pair_ids // topk
        cnt = len(tok_ids)

        xg = np.zeros((C, D), dtype=np.float32)
        xg[:cnt] = x[tok_ids]
        xT = np.ascontiguousarray(xg.T).reshape(KD, P, C)

        A4 = w13[ei].reshape(NI2, P, KD, P)          # [n, c, k, p]
        w13t = np.ascontiguousarray(A4.transpose(0, 3, 2, 1)).reshape(NI2, P, KD * P)
        B4 = w2[ei].reshape(ND, P, KI, P)            # [d, c, ki, p]
        w2t = np.ascontiguousarray(B4.transpose(0, 3, 2, 1)).reshape(ND, P, KI * P)

        in_maps.append({"xT": xT, "w13t": w13t, "w2t": w2t})

    trace = bool(os.environ.get("BASS_TRACE"))
    if trace:
        _ensure_ntff_hook()
    res = run_bass_kernel_spmd(nc, in_maps, core_ids=list(range(E)), trace=trace)
    LAST_EXEC_TIME_NS = res.exec_time_ns

    # ---- host scatter: un-permute per-expert outputs back to (token, slot) ----
    out_pairs = np.empty((M * TOPK, D), dtype=np.float32)
    for ei in range(E):
        pair_ids = order[starts[ei] : starts[ei + 1]]
        outT = res.results[ei]["outT"].reshape(D, C)
        out_pairs[pair_ids] = outT[:, : len(pair_ids)].T

    del out_idx_dtype
    return out_pairs.reshape(M, TOPK, D)


# revision 11
# speedup vs baseline: 1.3213x; 1.1768x over previous
"""Trainium2 Bass kernel for ConditionalFeedForward (MoE routed FFN).

Problem: M=2048 tokens, D=1024, I=2048, E=8 experts, TOPK=2.
out[t, s] = FFN_{e}(x[t]) with e = expert_indices[t, s], where
FFN_e(x) = (silu(x @ w1_e.T) * (x @ w3_e.T)) @ w2_e.T  (w13 = [w1; w3]).

Strategy (expert parallelism, 8 experts -> 8 cores):
 - Host routes (token, slot) pairs to the core owning the selected expert,
   pads each core's token batch to a common capacity C, and transposes
   activations so features live on SBUF partitions and tokens on the free
   dim.  No device collectives needed: the "all-to-all" is a host gather
   and scatter around one SPMD kernel launch.
 - Per core: hT = w13_e @ x_eT via PE (fp32r, full rate at free dim >=256),
   g = silu(h1) * h3 on ACT+DVE, outT = w2_e.T-contract on PE, DMA out.
 - Weights stream through SBUF once (24 MB/core), pre-tiled on the host
   into the exact [128, k*128] lhsT layout the tensor engine wants.
"""

import os

import numpy as np

import concourse.bass as bass
import concourse.tile as tile
from concourse import bacc, mybir
from concourse.bass_utils import run_bass_kernel_spmd

M, D, I, E, TOPK = 2048, 1024, 2048, 8, 2
P = 128
KD = D // P            # 8   k-tiles over D (mm1 contraction)
NI2 = (2 * I) // P     # 32  n-tiles over 2I (mm1 output rows)
NPAIR = NI2 // 2       # 16  (x1, x3) pairs
KI = I // P            # 16  k-tiles over I (mm2 contraction)
ND = D // P            # 8   d-tiles over D (mm2 output rows)

F32 = mybir.dt.float32
F32R = mybir.dt.float32r

# exec time of the most recent launch (ns), populated when BASS_TRACE=1
LAST_EXEC_TIME_NS = None

_program_cache = {}


def _chunks_for(C):
    """Split C token-columns into matmul moving-dim chunks (<=512 each)."""
    n_ch = -(-C // 512)
    base = -(-C // (n_ch * 32)) * 32
    chunks = []
    off = 0
    while off < C:
        cn = min(base, C - off)
        chunks.append((off, cn))
        off += cn
    return tuple(chunks)


def _build_program(C):
    chunks = _chunks_for(C)
    nc = bacc.Bacc("TRN2", target_bir_lowering=False, debug=False, num_devices=E)

    xT_d = nc.dram_tensor("xT", (KD, P, C), F32R, kind="ExternalInput").ap()
    w13_d = nc.dram_tensor("w13t", (NI2, P, KD * P), F32R, kind="ExternalInput").ap()
    w2_d = nc.dram_tensor("w2t", (ND, P, KI * P), F32R, kind="ExternalInput").ap()
    out_d = nc.dram_tensor("outT", (ND, P, C), F32, kind="ExternalOutput").ap()

    with tile.TileContext(nc) as tc:
        with (
            tc.tile_pool(name="xg", bufs=1) as xg_pool,
            tc.tile_pool(name="wt", bufs=3) as w_pool,
            tc.tile_pool(name="tmp", bufs=4) as tmp_pool,
            tc.tile_pool(name="ps", bufs=8, space="PSUM") as ps_pool,
        ):
            W13_BUFS = 6
            W13_AHEAD = W13_BUFS // 2
            w13_buf = {}

            def issue_w13(pr):
                wA = w_pool.tile(
                    [P, KD * P], F32R, tag="w13", name="w13", bufs=W13_BUFS
                )
                nc.sync.dma_start(wA[:], w13_d[pr])
                wB = w_pool.tile(
                    [P, KD * P], F32R, tag="w13", name="w13", bufs=W13_BUFS
                )
                nc.sync.dma_start(wB[:], w13_d[pr + NPAIR])
                w13_buf[pr] = (wA, wB)

            W2_BUFS = 3
            w2_buf = {}

            def issue_w2(d):
                wD = w_pool.tile([P, KI * P], F32R, tag="w2", name="w2", bufs=W2_BUFS)
                nc.sync.dma_start(wD[:], w2_d[d])
                w2_buf[d] = wD

            # startup order: the first matmul only needs x[0] and pair-0 wA,
            # so those two DMAs go first; the rest stream behind them.
            x_tiles = [
                xg_pool.tile([P, C], F32R, tag=f"x{k}", name=f"x{k}")
                for k in range(KD)
            ]
            nc.sync.dma_start(x_tiles[0][:], xT_d[0])
            wA0 = w_pool.tile([P, KD * P], F32R, tag="w13", name="w13", bufs=W13_BUFS)
            nc.sync.dma_start(wA0[:], w13_d[0])
            nc.sync.dma_start(x_tiles[1][:], xT_d[1])
            wB0 = w_pool.tile([P, KD * P], F32R, tag="w13", name="w13", bufs=W13_BUFS)
            nc.sync.dma_start(wB0[:], w13_d[NPAIR])
            w13_buf[0] = (wA0, wB0)
            for k in range(2, KD):
                nc.sync.dma_start(x_tiles[k][:], xT_d[k])
            for pr in range(1, 1 + W13_AHEAD - 1):
                issue_w13(pr)

            g_tiles = [
                xg_pool.tile([P, C], F32R, tag=f"g{ki}", name=f"g{ki}")
                for ki in range(KI)
            ]

            # ---- mm1 + silu*gate: process (x1, x3) row-block pairs ----
            for pr in range(NPAIR):
                nxt = pr + W13_AHEAD
                if nxt < NPAIR:
                    issue_w13(nxt)
                elif nxt - NPAIR < min(W2_BUFS, ND):
                    issue_w2(nxt - NPAIR)
                wA, wB = w13_buf.pop(pr)
                for c0, cn in chunks:
                    psA = ps_pool.tile([P, 512], F32, tag="ps", name="ps")[:, :cn]
                    psB = ps_pool.tile([P, 512], F32, tag="ps", name="ps")[:, :cn]
                    for k in range(KD):
                        nc.tensor.matmul(
                            psA,
                            wA[:, k * P : (k + 1) * P],
                            x_tiles[k][:, c0 : c0 + cn],
                            start=(k == 0),
                            stop=(k == KD - 1),
                        )
                    for k in range(KD):
                        nc.tensor.matmul(
                            psB,
                            wB[:, k * P : (k + 1) * P],
                            x_tiles[k][:, c0 : c0 + cn],
                            start=(k == 0),
                            stop=(k == KD - 1),
                        )
                    s = tmp_pool.tile([P, 512], F32, tag="s", name="s")[:, :cn]
                    nc.scalar.activation(s, psA, mybir.ActivationFunctionType.Silu)
                    nc.vector.tensor_mul(
                        out=g_tiles[pr][:, c0 : c0 + cn],
                        in0=s,
                        in1=psB,
                    )

            # ---- mm2: outT[d-block] = sum_ki w2T-tile @ g ----
            for d in range(ND):
                if d + W2_BUFS < ND:
                    issue_w2(d + W2_BUFS)
                wD = w2_buf.pop(d)
                for c0, cn in chunks:
                    psO = ps_pool.tile([P, 512], F32, tag="ps", name="ps")[:, :cn]
                    for ki in range(KI):
                        nc.tensor.matmul(
                            psO,
                            wD[:, ki * P : (ki + 1) * P],
                            g_tiles[ki][:, c0 : c0 + cn],
                            start=(ki == 0),
                            stop=(ki == KI - 1),
                        )
                    ot = tmp_pool.tile([P, 512], F32, tag="o", name="o")[:, :cn]
                    nc.vector.tensor_copy(ot, psO)
                    nc.sync.dma_start(out_d[d][:, c0 : c0 + cn], ot)

    nc.compile()
    return nc


def _get_program(C):
    if C not in _program_cache:
        _program_cache[C] = _build_program(C)
    return _program_cache[C]


def _ensure_ntff_hook():
    """Provide antenv.axon_hooks if the image lacks it, so trace=True works."""
    import sys
    import types

    try:
        import antenv.axon_hooks  # noqa: F401

        return
    except ImportError:
        pass
    try:
        import antenv
        from trn_agent_boot.trn_boot import _ntff_profile_via_ctypes

        mod = types.ModuleType("antenv.axon_hooks")
        state = {"hook": None}
        mod.set_axon_ntff_profile_hook = lambda h: state.__setitem__("hook", h)
        mod.get_axon_ntff_profile_hook = lambda: state["hook"]
        sys.modules["antenv.axon_hooks"] = mod
        antenv.axon_hooks = mod
        mod.set_axon_ntff_profile_hook(
            _ntff_profile_via_ctypes("/opt/axon/libaxon_pjrt.so")
        )
    except Exception:
        pass


def kernel(x, w13, w2, expert_indices):
    global LAST_EXEC_TIME_NS
    x = np.asarray(x, dtype=np.float32)
    w13 = np.asarray(w13, dtype=np.float32)
    w2 = np.asarray(w2, dtype=np.float32)
    idx = np.asarray(expert_indices)
    out_idx_dtype = idx.dtype
    idx32 = idx.astype(np.int64)

    m, d_model = x.shape
    e, two_i, _ = w13.shape
    inter = w2.shape[2]
    topk = idx.shape[1]
    assert (m, d_model, e, two_i, inter, topk) == (M, D, E, 2 * I, I, TOPK)

    # ---- host routing: group (token, slot) pairs by expert ----
    flat_e = idx32.reshape(-1)                       # [M*TOPK]
    order = np.argsort(flat_e, kind="stable")        # pair ids grouped by expert
    counts = np.bincount(flat_e, minlength=E)
    starts = np.concatenate([[0], np.cumsum(counts)])
    C = max(288, int(-(-counts.max() // 32) * 32))

    nc = _get_program(C)

    in_maps = []
    for ei in range(E):
        pair_ids = order[starts[ei] : starts[ei + 1]]
        tok_ids = pair_ids // topk
        cnt = len(tok_ids)

        xg = np.zeros((C, D), dtype=np.float32)
        xg[:cnt] = x[tok_ids]
        xT = np.ascontiguousarray(xg.T).reshape(KD, P, C)

        A4 = w13[ei].reshape(NI2, P, KD, P)          # [n, c, k, p]
        w13t = np.ascontiguousarray(A4.transpose(0, 3, 2, 1)).reshape(NI2, P, KD * P)
        B4 = w2[ei].reshape(ND, P, KI, P)            # [d, c, ki, p]
        w2t = np.ascontiguousarray(B4.transpose(0, 3, 2, 1)).reshape(ND, P, KI * P)

        in_maps.append({"xT": xT, "w13t": w13t, "w2t": w2t})

    trace = bool(os.environ.get("BASS_TRACE"))
    if trace:
        _ensure_ntff_hook()
    res = run_bass_kernel_spmd(nc, in_maps, core_ids=list(range(E)), trace=trace)
    LAST_EXEC_TIME_NS = res.exec_time_ns

    # ---- host scatter: un-permute per-expert outputs back to (token, slot) ----
    out_pairs = np.empty((M * TOPK, D), dtype=np.float32)
    for ei in range(E):
        pair_ids = order[starts[ei] : starts[ei + 1]]
        outT = res.results[ei]["outT"].reshape(D, C)
        out_pairs[pair_ids] = outT[:, : len(pair_ids)].T

    del out_idx_dtype
    return out_pairs.reshape(M, TOPK, D)


# revision 13
# speedup vs baseline: 1.3890x; 1.0512x over previous
"""Trainium2 Bass kernel for ConditionalFeedForward (MoE routed FFN).

Problem: M=2048 tokens, D=1024, I=2048, E=8 experts, TOPK=2.
out[t, s] = FFN_{e}(x[t]) with e = expert_indices[t, s], where
FFN_e(x) = (silu(x @ w1_e.T) * (x @ w3_e.T)) @ w2_e.T  (w13 = [w1; w3]).

Strategy (expert parallelism, 8 experts -> 8 cores):
 - Host routes (token, slot) pairs to the core owning the selected expert,
   pads each core's token batch to a common capacity C, and transposes
   activations so features live on SBUF partitions and tokens on the free
   dim.  No device collectives needed: the "all-to-all" is a host gather
   and scatter around one SPMD kernel launch.
 - Per core: hT = w13_e @ x_eT via PE (fp32r, full rate at free dim >=256),
   g = silu(h1) * h3 on ACT+DVE, outT = w2_e.T-contract on PE, DMA out.
 - Weights stream through SBUF once (24 MB/core), pre-tiled on the host
   into the exact [128, k*128] lhsT layout the tensor engine wants.
"""

import os

import numpy as np

import concourse.bass as bass
import concourse.tile as tile
from concourse import bacc, mybir
from concourse.bass_utils import run_bass_kernel_spmd

M, D, I, E, TOPK = 2048, 1024, 2048, 8, 2
P = 128
KD = D // P            # 8   k-tiles over D (mm1 contraction)
NI2 = (2 * I) // P     # 32  n-tiles over 2I (mm1 output rows)
NPAIR = NI2 // 2       # 16  (x1, x3) pairs
KI = I // P            # 16  k-tiles over I (mm2 contraction)
ND = D // P            # 8   d-tiles over D (mm2 output rows)

F32 = mybir.dt.float32
F32R = mybir.dt.float32r

# exec time of the most recent launch (ns), populated when BASS_TRACE=1
LAST_EXEC_TIME_NS = None

_program_cache = {}


def _chunks_for(C):
    """Split C token-columns into matmul moving-dim chunks (<=512 each)."""
    n_ch = -(-C // 512)
    base = -(-C // (n_ch * 32)) * 32
    chunks = []
    off = 0
    while off < C:
        cn = min(base, C - off)
        chunks.append((off, cn))
        off += cn
    return tuple(chunks)


def _build_program(C):
    chunks = _chunks_for(C)
    nc = bacc.Bacc(
        "TRN2",
        target_bir_lowering=False,
        debug=False,
        enable_asserts=False,
        num_devices=E,
    )

    xT_d = nc.dram_tensor("xT", (KD, P, C), F32R, kind="ExternalInput").ap()
    w13_d = nc.dram_tensor("w13t", (NI2, P, KD * P), F32R, kind="ExternalInput").ap()
    w2_d = nc.dram_tensor("w2t", (ND, P, KI * P), F32R, kind="ExternalInput").ap()
    out_d = nc.dram_tensor("outT", (ND, P, C), F32, kind="ExternalOutput").ap()

    with tile.TileContext(nc) as tc:
        with (
            tc.tile_pool(name="xg", bufs=1) as xg_pool,
            tc.tile_pool(name="wt", bufs=3) as w_pool,
            tc.tile_pool(name="tmp", bufs=4) as tmp_pool,
            tc.tile_pool(name="ps", bufs=8, space="PSUM") as ps_pool,
        ):
            W13_BUFS = 6
            W13_AHEAD = W13_BUFS // 2
            w13_buf = {}

            def issue_w13(pr):
                wA = w_pool.tile(
                    [P, KD * P], F32R, tag="w13", name="w13", bufs=W13_BUFS
                )
                nc.sync.dma_start(wA[:], w13_d[pr])
                wB = w_pool.tile(
                    [P, KD * P], F32R, tag="w13", name="w13", bufs=W13_BUFS
                )
                nc.sync.dma_start(wB[:], w13_d[pr + NPAIR])
                w13_buf[pr] = (wA, wB)

            W2_BUFS = 3
            w2_buf = {}

            def issue_w2(d):
                wD = w_pool.tile([P, KI * P], F32R, tag="w2", name="w2", bufs=W2_BUFS)
                nc.sync.dma_start(wD[:], w2_d[d])
                w2_buf[d] = wD

            # startup order: the first matmul only needs x[0] and pair-0 wA,
            # so those two DMAs go first; the rest stream behind them.
            x_tiles = [
                xg_pool.tile([P, C], F32R, tag=f"x{k}", name=f"x{k}")
                for k in range(KD)
            ]
            nc.sync.dma_start(x_tiles[0][:], xT_d[0])
            wA0 = w_pool.tile([P, KD * P], F32R, tag="w13", name="w13", bufs=W13_BUFS)
            nc.sync.dma_start(wA0[:], w13_d[0])
            nc.sync.dma_start(x_tiles[1][:], xT_d[1])
            wB0 = w_pool.tile([P, KD * P], F32R, tag="w13", name="w13", bufs=W13_BUFS)
            nc.sync.dma_start(wB0[:], w13_d[NPAIR])
            w13_buf[0] = (wA0, wB0)
            for k in range(2, KD):
                nc.sync.dma_start(x_tiles[k][:], xT_d[k])
            for pr in range(1, 1 + W13_AHEAD - 1):
                issue_w13(pr)

            g_tiles = [
                xg_pool.tile([P, C], F32R, tag=f"g{ki}", name=f"g{ki}")
                for ki in range(KI)
            ]

            # ---- mm1 + silu*gate: process (x1, x3) row-block pairs ----
            for pr in range(NPAIR):
                nxt = pr + W13_AHEAD
                if nxt < NPAIR:
                    issue_w13(nxt)
                elif nxt - NPAIR < min(W2_BUFS, ND):
                    issue_w2(nxt - NPAIR)
                wA, wB = w13_buf.pop(pr)
                for c0, cn in chunks:
                    psA = ps_pool.tile([P, 512], F32, tag="ps", name="ps")[:, :cn]
                    psB = ps_pool.tile([P, 512], F32, tag="ps", name="ps")[:, :cn]
                    for k in range(KD):
                        nc.tensor.matmul(
                            psA,
                            wA[:, k * P : (k + 1) * P],
                            x_tiles[k][:, c0 : c0 + cn],
                            start=(k == 0),
                            stop=(k == KD - 1),
                        )
                    for k in range(KD):
                        nc.tensor.matmul(
                            psB,
                            wB[:, k * P : (k + 1) * P],
                            x_tiles[k][:, c0 : c0 + cn],
                            start=(k == 0),
                            stop=(k == KD - 1),
                        )
                    s = tmp_pool.tile([P, 512], F32, tag="s", name="s")[:, :cn]
                    nc.scalar.activation(s, psA, mybir.ActivationFunctionType.Silu)
                    nc.vector.tensor_mul(
                        out=g_tiles[pr][:, c0 : c0 + cn],
                        in0=s,
                        in1=psB,
                    )

            # ---- mm2: outT[d-block] = sum_ki w2T-tile @ g ----
            for d in range(ND):
                if d + W2_BUFS < ND:
                    issue_w2(d + W2_BUFS)
                wD = w2_buf.pop(d)
                for c0, cn in chunks:
                    psO = ps_pool.tile([P, 512], F32, tag="ps", name="ps")[:, :cn]
                    for ki in range(KI):
                        nc.tensor.matmul(
                            psO,
                            wD[:, ki * P : (ki + 1) * P],
                            g_tiles[ki][:, c0 : c0 + cn],
                            start=(ki == 0),
                            stop=(ki == KI - 1),
                        )
                    ot = tmp_pool.tile([P, 512], F32, tag="o", name="o")[:, :cn]
                    nc.vector.tensor_copy(ot, psO)
                    nc.sync.dma_start(out_d[d][:, c0 : c0 + cn], ot)

    nc.compile()
    return nc


def _get_program(C):
    if C not in _program_cache:
        _program_cache[C] = _build_program(C)
    return _program_cache[C]


def _ensure_ntff_hook():
    """Provide antenv.axon_hooks if the image lacks it, so trace=True works."""
    import sys
    import types

    try:
        import antenv.axon_hooks  # noqa: F401

        return
    except ImportError:
        pass
    try:
        import antenv
        from trn_agent_boot.trn_boot import _ntff_profile_via_ctypes

        mod = types.ModuleType("antenv.axon_hooks")
        state = {"hook": None}
        mod.set_axon_ntff_profile_hook = lambda h: state.__setitem__("hook", h)
        mod.get_axon_ntff_profile_hook = lambda: state["hook"]
        sys.modules["antenv.axon_hooks"] = mod
        antenv.axon_hooks = mod
        mod.set_axon_ntff_profile_hook(
            _ntff_profile_via_ctypes("/opt/axon/libaxon_pjrt.so")
        )
    except Exception:
        pass


def kernel(x, w13, w2, expert_indices):
    global LAST_EXEC_TIME_NS
    x = np.asarray(x, dtype=np.float32)
    w13 = np.asarray(w13, dtype=np.float32)
    w2 = np.asarray(w2, dtype=np.float32)
    idx = np.asarray(expert_indices)
    out_idx_dtype = idx.dtype
    idx32 = idx.astype(np.int64)

    m, d_model = x.shape
    e, two_i, _ = w13.shape
    inter = w2.shape[2]
    topk = idx.shape[1]
    assert (m, d_model, e, two_i, inter, topk) == (M, D, E, 2 * I, I, TOPK)

    # ---- host routing: unique (token, expert) work items per expert ----
    # A token picking the same expert in both slots computes the FFN once;
    # the result is scattered to every matching slot.
    tok_unique = [
        np.unique(np.concatenate([np.nonzero(idx32[:, s] == ei)[0] for s in range(topk)]))
        for ei in range(E)
    ]
    max_cnt = max(len(u) for u in tok_unique)
    C = max(256, int(-(-max_cnt // 8) * 8))

    nc = _get_program(C)

    in_maps = []
    for ei in range(E):
        tok_ids = tok_unique[ei]
        cnt = len(tok_ids)

        xg = np.zeros((C, D), dtype=np.float32)
        xg[:cnt] = x[tok_ids]
        xT = np.ascontiguousarray(xg.T).reshape(KD, P, C)

        A4 = w13[ei].reshape(NI2, P, KD, P)          # [n, c, k, p]
        w13t = np.ascontiguousarray(A4.transpose(0, 3, 2, 1)).reshape(NI2, P, KD * P)
        B4 = w2[ei].reshape(ND, P, KI, P)            # [d, c, ki, p]
        w2t = np.ascontiguousarray(B4.transpose(0, 3, 2, 1)).reshape(ND, P, KI * P)

        in_maps.append({"xT": xT, "w13t": w13t, "w2t": w2t})

    trace = bool(os.environ.get("BASS_TRACE"))
    if trace:
        _ensure_ntff_hook()
    res = run_bass_kernel_spmd(nc, in_maps, core_ids=list(range(E)), trace=trace)
    LAST_EXEC_TIME_NS = res.exec_time_ns

    # ---- host scatter: copy each expert's outputs to all matching slots ----
    out = np.empty((M, topk, D), dtype=np.float32)
    for ei in range(E):
        outT = res.results[ei]["outT"].reshape(D, C)
        oe = outT[:, : len(tok_unique[ei])].T        # [cnt, D]
        for s in range(topk):
            sel = np.nonzero(idx32[:, s] == ei)[0]
            out[sel, s] = oe[np.searchsorted(tok_unique[ei], sel)]

    del out_idx_dtype
    return out


# revision 14
# speedup vs baseline: 1.4200x; 1.0223x over previous
"""Trainium2 Bass kernel for ConditionalFeedForward (MoE routed FFN).

Problem: M=2048 tokens, D=1024, I=2048, E=8 experts, TOPK=2.
out[t, s] = FFN_{e}(x[t]) with e = expert_indices[t, s], where
FFN_e(x) = (silu(x @ w1_e.T) * (x @ w3_e.T)) @ w2_e.T  (w13 = [w1; w3]).

Strategy (expert parallelism, 8 experts -> 8 cores):
 - Host routes (token, slot) pairs to the core owning the selected expert,
   pads each core's token batch to a common capacity C, and transposes
   activations so features live on SBUF partitions and tokens on the free
   dim.  No device collectives needed: the "all-to-all" is a host gather
   and scatter around one SPMD kernel launch.
 - Per core: hT = w13_e @ x_eT via PE (fp32r, full rate at free dim >=256),
   g = silu(h1) * h3 on ACT+DVE, outT = w2_e.T-contract on PE, DMA out.
 - Weights stream through SBUF once (24 MB/core), pre-tiled on the host
   into the exact [128, k*128] lhsT layout the tensor engine wants.
"""

import os

import numpy as np

import concourse.bass as bass
import concourse.tile as tile
from concourse import bacc, mybir
from concourse.bass_utils import run_bass_kernel_spmd

M, D, I, E, TOPK = 2048, 1024, 2048, 8, 2
P = 128
KD = D // P            # 8   k-tiles over D (mm1 contraction)
NI2 = (2 * I) // P     # 32  n-tiles over 2I (mm1 output rows)
NPAIR = NI2 // 2       # 16  (x1, x3) pairs
KI = I // P            # 16  k-tiles over I (mm2 contraction)
ND = D // P            # 8   d-tiles over D (mm2 output rows)

F32 = mybir.dt.float32
F32R = mybir.dt.float32r

# exec time of the most recent launch (ns), populated when BASS_TRACE=1
LAST_EXEC_TIME_NS = None

_program_cache = {}


def _chunks_for(C):
    """Split C token-columns into matmul moving-dim chunks (<=512 each)."""
    n_ch = -(-C // 512)
    base = -(-C // (n_ch * 32)) * 32
    chunks = []
    off = 0
    while off < C:
        cn = min(base, C - off)
        chunks.append((off, cn))
        off += cn
    return tuple(chunks)


def _build_program(C):
    chunks = _chunks_for(C)
    nc = bacc.Bacc(
        "TRN2",
        target_bir_lowering=False,
        debug=False,
        enable_asserts=False,
        num_devices=E,
    )

    xT_d = nc.dram_tensor("xT", (KD, P, C), F32R, kind="ExternalInput").ap()
    w13_d = nc.dram_tensor("w13t", (NI2, P, KD * P), F32R, kind="ExternalInput").ap()
    w2_d = nc.dram_tensor("w2t", (ND, P, KI * P), F32R, kind="ExternalInput").ap()
    out_d = nc.dram_tensor("outT", (ND, P, C), F32, kind="ExternalOutput").ap()

    with tile.TileContext(nc) as tc:
        with (
            tc.tile_pool(name="xg", bufs=1) as xg_pool,
            tc.tile_pool(name="wt", bufs=3) as w_pool,
            tc.tile_pool(name="tmp", bufs=4) as tmp_pool,
            tc.tile_pool(name="ps", bufs=8, space="PSUM") as ps_pool,
        ):
            W13_BUFS = 8
            W13_AHEAD = W13_BUFS // 2
            w13_buf = {}

            def issue_w13(pr):
                wA = w_pool.tile(
                    [P, KD * P], F32R, tag="w13", name="w13", bufs=W13_BUFS
                )
                nc.sync.dma_start(wA[:], w13_d[pr])
                wB = w_pool.tile(
                    [P, KD * P], F32R, tag="w13", name="w13", bufs=W13_BUFS
                )
                nc.sync.dma_start(wB[:], w13_d[pr + NPAIR])
                w13_buf[pr] = (wA, wB)

            W2_BUFS = 4
            w2_buf = {}

            def issue_w2(d):
                wD = w_pool.tile([P, KI * P], F32R, tag="w2", name="w2", bufs=W2_BUFS)
                nc.sync.dma_start(wD[:], w2_d[d])
                w2_buf[d] = wD

            # startup order: the first matmul only needs x[0] and pair-0 wA,
            # so those two DMAs go first; the rest stream behind them.
            x_tiles = [
                xg_pool.tile([P, C], F32R, tag=f"x{k}", name=f"x{k}")
                for k in range(KD)
            ]
            nc.sync.dma_start(x_tiles[0][:], xT_d[0])
            wA0 = w_pool.tile([P, KD * P], F32R, tag="w13", name="w13", bufs=W13_BUFS)
            nc.sync.dma_start(wA0[:], w13_d[0])
            nc.sync.dma_start(x_tiles[1][:], xT_d[1])
            wB0 = w_pool.tile([P, KD * P], F32R, tag="w13", name="w13", bufs=W13_BUFS)
            nc.sync.dma_start(wB0[:], w13_d[NPAIR])
            w13_buf[0] = (wA0, wB0)
            for k in range(2, KD):
                nc.sync.dma_start(x_tiles[k][:], xT_d[k])
            for pr in range(1, 1 + W13_AHEAD - 1):
                issue_w13(pr)

            g_tiles = [
                xg_pool.tile([P, C], F32R, tag=f"g{ki}", name=f"g{ki}")
                for ki in range(KI)
            ]

            # ---- mm1 + silu*gate: process (x1, x3) row-block pairs ----
            for pr in range(NPAIR):
                nxt = pr + W13_AHEAD
                if nxt < NPAIR:
                    issue_w13(nxt)
                elif nxt - NPAIR < min(W2_BUFS, ND):
                    issue_w2(nxt - NPAIR)
                wA, wB = w13_buf.pop(pr)
                for c0, cn in chunks:
                    psA = ps_pool.tile([P, 512], F32, tag="ps", name="ps")[:, :cn]
                    psB = ps_pool.tile([P, 512], F32, tag="ps", name="ps")[:, :cn]
                    for k in range(KD):
                        nc.tensor.matmul(
                            psA,
                            wA[:, k * P : (k + 1) * P],
                            x_tiles[k][:, c0 : c0 + cn],
                            start=(k == 0),
                            stop=(k == KD - 1),
                        )
                    for k in range(KD):
                        nc.tensor.matmul(
                            psB,
                            wB[:, k * P : (k + 1) * P],
                            x_tiles[k][:, c0 : c0 + cn],
                            start=(k == 0),
                            stop=(k == KD - 1),
                        )
                    s = tmp_pool.tile([P, 512], F32, tag="s", name="s")[:, :cn]
                    nc.scalar.activation(s, psA, mybir.ActivationFunctionType.Silu)
                    nc.vector.tensor_mul(
                        out=g_tiles[pr][:, c0 : c0 + cn],
                        in0=s,
                        in1=psB,
                    )

            # ---- mm2: outT[d-block] = sum_ki w2T-tile @ g ----
            for d in range(ND):
                if d + W2_BUFS < ND:
                    issue_w2(d + W2_BUFS)
                wD = w2_buf.pop(d)
                for c0, cn in chunks:
                    psO = ps_pool.tile([P, 512], F32, tag="ps", name="ps")[:, :cn]
                    for ki in range(KI):
                        nc.tensor.matmul(
                            psO,
                            wD[:, ki * P : (ki + 1) * P],
                            g_tiles[ki][:, c0 : c0 + cn],
                            start=(ki == 0),
                            stop=(ki == KI - 1),
                        )
                    ot = tmp_pool.tile([P, 512], F32, tag="o", name="o")[:, :cn]
                    nc.vector.tensor_copy(ot, psO)
                    nc.sync.dma_start(out_d[d][:, c0 : c0 + cn], ot)

    nc.compile()
    return nc


def _get_program(C):
    if C not in _program_cache:
        _program_cache[C] = _build_program(C)
    return _program_cache[C]


def _ensure_ntff_hook():
    """Provide antenv.axon_hooks if the image lacks it, so trace=True works."""
    import sys
    import types

    try:
        import antenv.axon_hooks  # noqa: F401

        return
    except ImportError:
        pass
    try:
        import antenv
        from trn_agent_boot.trn_boot import _ntff_profile_via_ctypes

        mod = types.ModuleType("antenv.axon_hooks")
        state = {"hook": None}
        mod.set_axon_ntff_profile_hook = lambda h: state.__setitem__("hook", h)
        mod.get_axon_ntff_profile_hook = lambda: state["hook"]
        sys.modules["antenv.axon_hooks"] = mod
        antenv.axon_hooks = mod
        mod.set_axon_ntff_profile_hook(
            _ntff_profile_via_ctypes("/opt/axon/libaxon_pjrt.so")
        )
    except Exception:
        pass


def kernel(x, w13, w2, expert_indices):
    global LAST_EXEC_TIME_NS
    x = np.asarray(x, dtype=np.float32)
    w13 = np.asarray(w13, dtype=np.float32)
    w2 = np.asarray(w2, dtype=np.float32)
    idx = np.asarray(expert_indices)
    out_idx_dtype = idx.dtype
    idx32 = idx.astype(np.int64)

    m, d_model = x.shape
    e, two_i, _ = w13.shape
    inter = w2.shape[2]
    topk = idx.shape[1]
    assert (m, d_model, e, two_i, inter, topk) == (M, D, E, 2 * I, I, TOPK)

    # ---- host routing: unique (token, expert) work items per expert ----
    # A token picking the same expert in both slots computes the FFN once;
    # the result is scattered to every matching slot.
    tok_unique = [
        np.unique(np.concatenate([np.nonzero(idx32[:, s] == ei)[0] for s in range(topk)]))
        for ei in range(E)
    ]
    max_cnt = max(len(u) for u in tok_unique)
    C = max(256, int(-(-max_cnt // 8) * 8))

    nc = _get_program(C)

    in_maps = []
    for ei in range(E):
        tok_ids = tok_unique[ei]
        cnt = len(tok_ids)

        xg = np.zeros((C, D), dtype=np.float32)
        xg[:cnt] = x[tok_ids]
        xT = np.ascontiguousarray(xg.T).reshape(KD, P, C)

        A4 = w13[ei].reshape(NI2, P, KD, P)          # [n, c, k, p]
        w13t = np.ascontiguousarray(A4.transpose(0, 3, 2, 1)).reshape(NI2, P, KD * P)
        B4 = w2[ei].reshape(ND, P, KI, P)            # [d, c, ki, p]
        w2t = np.ascontiguousarray(B4.transpose(0, 3, 2, 1)).reshape(ND, P, KI * P)

        in_maps.append({"xT": xT, "w13t": w13t, "w2t": w2t})

    trace = bool(os.environ.get("BASS_TRACE"))
    if trace:
        _ensure_ntff_hook()
    res = run_bass_kernel_spmd(nc, in_maps, core_ids=list(range(E)), trace=trace)
    LAST_EXEC_TIME_NS = res.exec_time_ns

    # ---- host scatter: copy each expert's outputs to all matching slots ----
    out = np.empty((M, topk, D), dtype=np.float32)
    for ei in range(E):
        outT = res.results[ei]["outT"].reshape(D, C)
        oe = outT[:, : len(tok_unique[ei])].T        # [cnt, D]
        for s in range(topk):
            sel = np.nonzero(idx32[:, s] == ei)[0]
            out[sel, s] = oe[np.searchsorted(tok_unique[ei], sel)]

    del out_idx_dtype
    return out
